# revision 24
# baseline (speedup 1.0000x reference)
"""Trainium2 Bass kernel for nn_AttentionMechanism (B=4, LQ=1024, ND=4096, D=1024).

Sharding: batch (4) x num_docs (2) -> 8 cores. Core c handles batch c//2 and
doc half c%2 (2048 docs).

Algebraic restructuring (exact up to float rounding):
  scores = (x@Wq.T + bq) @ (docs@Wk.T + bk).T
         = x @ (Wq.T@Wk) @ docs.T + [x@(Wq.T@bk)]_per-query + [docs@(Wk.T@bq)]_per-doc + bq.bk
Softmax over docs is invariant to per-query constants, so only
  scores' = x @ Wqk @ docs.T + t3[n],   Wqk = Wq.T@Wk (host),  t3 = docs @ (Wk.T@bq)
is needed — the K-projection (the largest matmul block) disappears entirely and
raw docs.T is the scores operand. Both per-core partials of a batch drop the
same per-query constants, so the host-side softmax-stat merge is unaffected.

Per core:
  aqT = Wqk.T-chunks @ queryT     [d', lq]  (fp32r, d' on partitions)
  t3b = broadcast(docs @ w)       [128, n]  (via replicated-w matmul)
  s   = aqT.T @ docsT + t3        [lq, n]   per 128-row chunk, PSUM
  m   = rowmax(s); p = exp(s - m); l = rowsum(p)
  num = p @ docs                  [lq, d]
Host merges the two doc-halves per batch (softmax-stat rescale) and divides.

All heavy matmuls run as float32r (TF32-like, full PE rate).
"""

import sys

if "/opt/trn_rl_repo" not in sys.path:
    sys.path.insert(0, "/opt/trn_rl_repo")

import numpy as np

import concourse.bass as bass  # noqa: F401
import concourse.mybir as mybir
from concourse import bacc
from concourse.tile import TileContext
from concourse.masks import make_identity
from concourse.bass_utils import run_bass_kernel_spmd

P = 128
B, LQ, ND, D = 4, 1024, 4096, 1024
N2 = ND // 2  # docs per core
EC = D // P  # 8 contraction chunks (d')
DC = D // P  # 8 contraction chunks (d)
LC = LQ // P  # 8 lq-chunks
NC = N2 // P  # 16 n-chunks
NT = N2 // 512  # 4 n-tiles of 512

F32 = mybir.dt.float32
F32R = mybir.dt.float32r
ACT = mybir.ActivationFunctionType
AX = mybir.AxisListType

_CACHE = {}


def build_nc():
    nc = bacc.Bacc("TRN2", target_bir_lowering=False)

    qT = nc.dram_tensor("qT", [D, LQ], F32, kind="ExternalInput")
    dT = nc.dram_tensor("dT", [D, N2], F32, kind="ExternalInput")
    dn = nc.dram_tensor("dn", [N2, D], F32, kind="ExternalInput")
    wqk = nc.dram_tensor("wqk", [D, D], F32, kind="ExternalInput")
    wrep = nc.dram_tensor("wrep", [P, DC, P], F32, kind="ExternalInput")

    num = nc.dram_tensor("num", [LQ, D], F32, kind="ExternalOutput")
    mx = nc.dram_tensor("mx", [P, LC], F32, kind="ExternalOutput")
    ls = nc.dram_tensor("ls", [P, LC], F32, kind="ExternalOutput")

    qT_r = qT.ap().rearrange("(dc p) l -> p dc l", p=P).bitcast(F32R)
    dT_r = dT.ap().rearrange("(dc p) n -> p dc n", p=P).bitcast(F32R)
    dn_r = dn.ap().rearrange("(nc p) d -> p nc d", p=P).bitcast(F32R)
    wqk_r = wqk.ap().rearrange("(dc p) e -> p dc e", p=P).bitcast(F32R)
    wrep_r = wrep.ap().bitcast(F32R)

    with TileContext(nc) as tc:
        with (
            tc.tile_pool(name="const", bufs=1) as cpool,
            tc.tile_pool(name="stats", bufs=1) as spool,
            tc.tile_pool(name="dTp", bufs=1) as dT_pool,
            tc.tile_pool(name="aqTp", bufs=1) as aqT_pool,
            tc.tile_pool(name="t3p", bufs=1) as t3_pool,
        ):
            ident32 = cpool.tile([P, P], F32)
            make_identity(nc, ident32[:])
            ident = cpool.tile([P, P], F32R)
            nc.vector.tensor_copy(ident[:], ident32[:])

            wrep_s = cpool.tile([P, DC, P], F32R)
            nc.sync.dma_start(wrep_s[:], wrep_r)

            mx_all = spool.tile([P, LC], F32)
            ls_all = spool.tile([P, LC], F32)

            aqT = [aqT_pool.tile([P, LQ], F32R, name=f"aqT{ec}") for ec in range(EC)]
            t3b = t3_pool.tile([P, N2], F32)
            dT_t = [dT_pool.tile([P, N2], F32R, name=f"dTt{dc}") for dc in range(DC)]

            # ---- Phase P: aqT[d', lq] = Wqk.T-chunks @ queryT; t3 row ----
            with (
                tc.tile_pool(name="pp", bufs=1) as pp,
                tc.tile_pool(name="psp", bufs=4, space="PSUM") as psp,
                tc.tile_pool(name="ps3", bufs=4, space="PSUM") as ps3,
            ):
                wqk_t, qT_t = [], []
                for dc in range(DC):
                    w = pp.tile([P, D], F32R, name=f"wqk{dc}")
                    q = pp.tile([P, LQ], F32R, name=f"qTt{dc}")
                    nc.sync.dma_start(w[:], wqk_r[:, dc, :])
                    nc.sync.dma_start(q[:], qT_r[:, dc, :])
                    wqk_t.append(w)
                    qT_t.append(q)

                for dc in range(DC):
                    nc.sync.dma_start(dT_t[dc][:], dT_r[:, dc, :])

                for ec in range(EC):
                    pss = [psp.tile([P, 512], F32, name="psp") for t in range(2)]
                    for dc in range(DC):
                        for t in range(2):
                            nc.tensor.matmul(
                                pss[t][:],
                                wqk_t[dc][:, ec * P : (ec + 1) * P],
                                qT_t[dc][:, t * 512 : (t + 1) * 512],
                                start=(dc == 0),
                                stop=(dc == DC - 1),
                            )
                    for t in range(2):
                        nc.scalar.activation(
                            aqT[ec][:, t * 512 : (t + 1) * 512],
                            pss[t][:],
                            ACT.Identity,
                            bias=0.0,
                        )

                # t3 broadcast row: every partition gets t3[n] (w replicated
                # as the stationary operand's columns)
                ps3t = [ps3.tile([P, 512], F32, name="ps3") for t in range(NT)]
                for dc in range(DC):
                    for t in range(NT):
                        nc.tensor.matmul(
                            ps3t[t][:],
                            wrep_s[:, dc, :],
                            dT_t[dc][:, t * 512 : (t + 1) * 512],
                            start=(dc == 0),
                            stop=(dc == DC - 1),
                        )
                for t in range(NT):
                    nc.scalar.activation(
                        t3b[:, t * 512 : (t + 1) * 512],
                        ps3t[t][:],
                        ACT.Copy,
                    )

            # ---- Phase A: attention per 128-query chunk ----
            with (
                tc.tile_pool(name="pa", bufs=1) as pa,
                tc.tile_pool(name="pwork", bufs=2) as pw,
                tc.tile_pool(name="pwork1", bufs=1) as pw1,
                tc.tile_pool(name="ps_sc", bufs=5, space="PSUM") as ps_sc,
                tc.tile_pool(name="ps_av", bufs=1, space="PSUM") as ps_av,
                tc.tile_pool(name="ps_tp", bufs=1, space="PSUM") as ps_tp,
            ):
                # dn loads on SWDGE (gpsimd) queues: keeps the sync queue
                # free so phase-A PE work isn't gated behind this drain.
                dn_s = []
                for i in range(NC):
                    t = pa.tile([P, D], F32R, name=f"dn{i}")
                    nc.gpsimd.dma_start(t[:], dn_r[:, i, :])
                    dn_s.append(t)

                # Software pipeline: the next chunk's score matmuls are
                # emitted into the softmax-latency stall of the current
                # chunk, using a 5-slot rotating score-PSUM pool.
                scs = {}
                mx4s = {}
                nm3s = {}

                def emit_scores_mm(lc, ts):
                    lq_sl = slice(lc * P, (lc + 1) * P)
                    if lc not in mx4s:
                        mx4s[lc] = pw.tile([P, NT], F32, name="mx4")
                    for ec in range(EC):
                        for t in ts:
                            if (lc, t) not in scs:
                                scs[(lc, t)] = ps_sc.tile([P, 512], F32, name="sc")
                            nc.tensor.matmul(
                                scs[(lc, t)][:],
                                aqT[ec][:, lq_sl],
                                dT_t[ec][:, t * 512 : (t + 1) * 512],
                                start=(ec == 0),
                                stop=(ec == EC - 1),
                            )

                def emit_scores_red(lc, ts):
                    for t in ts:
                        # add the per-doc bias row, then rowmax
                        nc.vector.tensor_tensor(
                            scs[(lc, t)][:],
                            scs[(lc, t)][:],
                            t3b[:, t * 512 : (t + 1) * 512],
                            mybir.AluOpType.add,
                        )
                        nc.vector.reduce_max(
                            mx4s[lc][:, t : t + 1], scs[(lc, t)][:], axis=AX.X
                        )
                    if ts[-1] == NT - 1:
                        # partial max over t0..2; final combine at chunk head
                        nm3 = pw.tile([P, 1], F32, name="nm3")
                        nc.vector.reduce_max(
                            nm3[:], mx4s[lc][:, 0 : NT - 1], axis=AX.X
                        )
                        nm3s[lc] = nm3

                def emit_scores(lc, ts):
                    emit_scores_mm(lc, ts)
                    emit_scores_red(lc, ts)

                emit_scores(0, [0, 1])
                emit_scores(0, [2, 3])
                for lc in range(LC):
                    lq_sl = slice(lc * P, (lc + 1) * P)
                    mx4 = mx4s.pop(lc)
                    nm3 = nm3s.pop(lc)
                    ls8 = pw.tile([P, 2 * NT], F32, name="ls8")
                    negmax = pw.tile([P, 1], F32, name="negmax")
                    nc.vector.tensor_tensor(
                        mx_all[:, lc : lc + 1],
                        nm3[:],
                        mx4[:, NT - 1 : NT],
                        mybir.AluOpType.max,
                    )
                    nc.vector.tensor_scalar_mul(
                        negmax[:], mx_all[:, lc : lc + 1], -1.0
                    )
                    if lc + 1 < LC:
                        emit_scores_mm(lc + 1, [0, 1])
                    # per 512-group: exp -> transpose -> AV, interleaved
                    av = ps_av.tile([P, D], F32, name="av")
                    for g in range(NT):
                        sc = scs.pop((lc, g))
                        probs_h = [
                            pw1.tile([P, 256], F32R, name=f"probs{g}_{h}")
                            for h in range(2)
                        ]
                        for h in range(2):
                            nc.scalar.activation(
                                probs_h[h][:],
                                sc[:, h * 256 : (h + 1) * 256],
                                ACT.Exp,
                                bias=negmax[:],
                                accum_out=ls8[:, 2 * g + h : 2 * g + h + 1],
                            )
                        tp = ps_tp.tile([P, 512], F32R, name="tp")
                        for j in range(4):
                            nc.tensor.transpose(
                                tp[:, j * P : (j + 1) * P],
                                probs_h[j // 2][:, (j % 2) * P : (j % 2 + 1) * P],
                                ident[:],
                            )
                        probsT = pw.tile([P, 4, P], F32R, name=f"probsT{g}")
                        nc.vector.tensor_copy(probsT[:], tp[:])
                        for j in range(4):
                            nn = g * 4 + j
                            for dh in range(2):
                                nc.tensor.matmul(
                                    av[:, dh * 512 : (dh + 1) * 512],
                                    probsT[:, j, :],
                                    dn_s[nn][:, dh * 512 : (dh + 1) * 512],
                                    start=(nn == 0),
                                    stop=(nn == NC - 1),
                                )
                        if lc + 1 < LC:
                            if g == 0:
                                emit_scores_red(lc + 1, [0, 1])
                                emit_scores_mm(lc + 1, [2, 3])
                            elif g == 2:
                                emit_scores_red(lc + 1, [2, 3])
                    nc.vector.reduce_sum(
                        ls_all[:, lc : lc + 1], ls8[:], axis=AX.X
                    )
                    num_t = pw1.tile([P, D], F32, name="num_t")
                    if lc == LC - 1:
                        for dh in range(2):
                            sl = slice(dh * 512, (dh + 1) * 512)
                            nc.scalar.activation(
                                num_t[:, sl], av[:, sl], ACT.Copy
                            )
                            nc.sync.dma_start(num.ap()[lq_sl, sl], num_t[:, sl])
                    else:
                        nc.scalar.activation(num_t[:], av[:], ACT.Copy)
                        nc.sync.dma_start(num.ap()[lq_sl, :], num_t[:])

            nc.sync.dma_start(mx.ap()[:, :], mx_all[:])
            nc.sync.dma_start(ls.ap()[:, :], ls_all[:])

    nc.compile()
    return nc


def _prep_inputs(query, documents, Wq, bq, Wk, bk):
    query = np.asarray(query, dtype=np.float32)
    documents = np.asarray(documents, dtype=np.float32)
    Wq64 = np.asarray(Wq, np.float64)
    Wk64 = np.asarray(Wk, np.float64)
    bq64 = np.asarray(bq, np.float64)
    wqk = np.ascontiguousarray((Wq64.T @ Wk64).astype(np.float32))
    w = (Wk64.T @ bq64).astype(np.float32)  # [D] per-doc bias vector
    wrep = np.ascontiguousarray(
        np.broadcast_to(w.reshape(DC, P).T[:, :, None], (P, DC, P))
    ).astype(np.float32)
    in_maps = []
    for b in range(B):
        qTh = np.ascontiguousarray(query[b].T)
        for h in range(2):
            d_slice = documents[b, h * N2 : (h + 1) * N2]
            in_maps.append(
                {
                    "qT": qTh,
                    "dT": np.ascontiguousarray(d_slice.T),
                    "dn": np.ascontiguousarray(d_slice),
                    "wqk": wqk,
                    "wrep": wrep,
                }
            )
    return in_maps


def _merge(results):
    out = np.empty((B, LQ, D), dtype=np.float32)
    for b in range(B):
        r0, r1 = results[2 * b], results[2 * b + 1]
        m0 = np.asarray(r0["mx"]).T.reshape(LQ).astype(np.float64)
        m1 = np.asarray(r1["mx"]).T.reshape(LQ).astype(np.float64)
        l0 = np.asarray(r0["ls"]).T.reshape(LQ).astype(np.float64)
        l1 = np.asarray(r1["ls"]).T.reshape(LQ).astype(np.float64)
        n0 = np.asarray(r0["num"]).astype(np.float64)
        n1 = np.asarray(r1["num"]).astype(np.float64)
        m = np.maximum(m0, m1)
        a0 = np.exp(m0 - m)
        a1 = np.exp(m1 - m)
        denom = a0 * l0 + a1 * l1
        out[b] = ((a0[:, None] * n0 + a1[:, None] * n1) / denom[:, None]).astype(
            np.float32
        )
    return out


def run(inputs, trace=False, trace_kwargs=None):
    """Run the SPMD kernel; returns (output, BassKernelResults)."""
    if "nc" not in _CACHE:
        _CACHE["nc"] = build_nc()
    nc = _CACHE["nc"]
    in_maps = _prep_inputs(**inputs)
    kw = {}
    if trace:
        kw["trace"] = True
        kw.update(trace_kwargs or {})
    res = run_bass_kernel_spmd(nc, in_maps, core_ids=list(range(8)), **kw)
    return _merge(res.results), res


def kernel(**inputs) -> np.ndarray:
    out, _ = run(inputs)
    return out


# revision 25
# speedup vs baseline: 1.1597x; 1.1597x over previous
"""Trainium2 Bass kernel for nn_AttentionMechanism (B=4, LQ=1024, ND=4096, D=1024).

Sharding: batch (4) x num_docs (2) -> 8 cores. Core c handles batch c//2 and
doc half c%2 (2048 docs).

Algebraic restructuring (exact up to float rounding):
  scores = (x@Wq.T + bq) @ (docs@Wk.T + bk).T
         = x @ (Wq.T@Wk) @ docs.T + [x@(Wq.T@bk)]_per-query + [docs@(Wk.T@bq)]_per-doc + bq.bk
Softmax over docs is invariant to per-query constants, so only
  scores' = x @ Wqk @ docs.T + t3[n],   Wqk = Wq.T@Wk (host),  t3 = docs @ (Wk.T@bq)
is needed — the K-projection (the largest matmul block) disappears entirely and
raw docs.T is the scores operand. Both per-core partials of a batch drop the
same per-query constants, so the host-side softmax-stat merge is unaffected.

Per core:
  aqT = Wqk.T-chunks @ queryT     [d', lq]  (fp32r, d' on partitions)
  t3b = broadcast(docs @ w)       [128, n]  (via replicated-w matmul)
  s   = aqT.T @ docsT + t3        [lq, n]   per 128-row chunk, PSUM
  m   = rowmax(s); p = exp(s - m); l = rowsum(p)
  num = p @ docs                  [lq, d]
Host merges the two doc-halves per batch (softmax-stat rescale) and divides.

All heavy matmuls run as float32r (TF32-like, full PE rate).
"""

import sys

if "/opt/trn_rl_repo" not in sys.path:
    sys.path.insert(0, "/opt/trn_rl_repo")

import numpy as np

import concourse.bass as bass  # noqa: F401
import concourse.mybir as mybir
from concourse import bacc
from concourse.tile import TileContext
from concourse.masks import make_identity
from concourse.bass_utils import run_bass_kernel_spmd

P = 128
B, LQ, ND, D = 4, 1024, 4096, 1024
N2 = ND // 2  # docs per core
EC = D // P  # 8 contraction chunks (d')
DC = D // P  # 8 contraction chunks (d)
LC = LQ // P  # 8 lq-chunks
NC = N2 // P  # 16 n-chunks
NT = N2 // 512  # 4 n-tiles of 512

F32 = mybir.dt.float32
F32R = mybir.dt.float32r
ACT = mybir.ActivationFunctionType
AX = mybir.AxisListType

_CACHE = {}


def build_nc():
    nc = bacc.Bacc("TRN2", target_bir_lowering=False)

    qT = nc.dram_tensor("qT", [D, LQ], F32, kind="ExternalInput")
    dT = nc.dram_tensor("dT", [D, N2], F32, kind="ExternalInput")
    dn = nc.dram_tensor("dn", [N2, D], F32, kind="ExternalInput")
    wqk = nc.dram_tensor("wqk", [D, D], F32, kind="ExternalInput")
    wrep = nc.dram_tensor("wrep", [P, DC, P], F32, kind="ExternalInput")

    num = nc.dram_tensor("num", [LQ, D], F32, kind="ExternalOutput")
    mx = nc.dram_tensor("mx", [P, LC], F32, kind="ExternalOutput")
    ls = nc.dram_tensor("ls", [P, LC], F32, kind="ExternalOutput")

    qT_r = qT.ap().rearrange("(dc p) l -> p dc l", p=P).bitcast(F32R)
    dT_r = dT.ap().rearrange("(dc p) n -> p dc n", p=P).bitcast(F32R)
    dn_r = dn.ap().rearrange("(nc p) d -> p nc d", p=P).bitcast(F32R)
    wqk_r = wqk.ap().rearrange("(dc p) e -> p dc e", p=P).bitcast(F32R)
    wrep_r = wrep.ap().bitcast(F32R)

    with TileContext(nc) as tc:
        with (
            tc.tile_pool(name="const", bufs=1) as cpool,
            tc.tile_pool(name="stats", bufs=1) as spool,
            tc.tile_pool(name="dTp", bufs=1) as dT_pool,
            tc.tile_pool(name="aqTp", bufs=1) as aqT_pool,
            tc.tile_pool(name="t3p", bufs=1) as t3_pool,
        ):
            ident32 = cpool.tile([P, P], F32)
            make_identity(nc, ident32[:])
            ident = cpool.tile([P, P], F32R)
            nc.vector.tensor_copy(ident[:], ident32[:])

            mx_all = spool.tile([P, LC], F32)
            ls_all = spool.tile([P, LC], F32)

            aqT = [aqT_pool.tile([P, LQ], F32R, name=f"aqT{ec}") for ec in range(EC)]
            t3b = t3_pool.tile([P, N2], F32)
            dT_t = [dT_pool.tile([P, N2], F32R, name=f"dTt{dc}") for dc in range(DC)]

            # ---- Phase P: aqT[d', lq] = Wqk.T-chunks @ queryT; t3 row ----
            with (
                tc.tile_pool(name="pp", bufs=1) as pp,
                tc.tile_pool(name="psp", bufs=4, space="PSUM") as psp,
                tc.tile_pool(name="ps3", bufs=4, space="PSUM") as ps3,
            ):
                wqk_t, qT_t = [], []
                for dc in range(DC):
                    w = pp.tile([P, D], F32R, name=f"wqk{dc}")
                    q = pp.tile([P, LQ], F32R, name=f"qTt{dc}")
                    nc.sync.dma_start(w[:], wqk_r[:, dc, :])
                    nc.sync.dma_start(q[:], qT_r[:, dc, :])
                    wqk_t.append(w)
                    qT_t.append(q)
                wrep_s = pp.tile([P, DC, P], F32R, name="wrep")
                nc.sync.dma_start(wrep_s[:], wrep_r)
                for dc in range(DC):
                    nc.sync.dma_start(dT_t[dc][:], dT_r[:, dc, :])

                for ec in range(EC):
                    pss = [psp.tile([P, 512], F32, name="psp") for t in range(2)]
                    for dc in range(DC):
                        for t in range(2):
                            nc.tensor.matmul(
                                pss[t][:],
                                wqk_t[dc][:, ec * P : (ec + 1) * P],
                                qT_t[dc][:, t * 512 : (t + 1) * 512],
                                start=(dc == 0),
                                stop=(dc == DC - 1),
                            )
                    for t in range(2):
                        nc.scalar.activation(
                            aqT[ec][:, t * 512 : (t + 1) * 512],
                            pss[t][:],
                            ACT.Identity,
                            bias=0.0,
                        )

                # t3 broadcast row: every partition gets t3[n] (w replicated
                # as the stationary operand's columns)
                ps3t = [ps3.tile([P, 512], F32, name="ps3") for t in range(NT)]
                for dc in range(DC):
                    for t in range(NT):
                        nc.tensor.matmul(
                            ps3t[t][:],
                            wrep_s[:, dc, :],
                            dT_t[dc][:, t * 512 : (t + 1) * 512],
                            start=(dc == 0),
                            stop=(dc == DC - 1),
                        )
                for t in range(NT):
                    nc.scalar.activation(
                        t3b[:, t * 512 : (t + 1) * 512],
                        ps3t[t][:],
                        ACT.Copy,
                    )

            # ---- Phase A: attention per 128-query chunk ----
            with (
                tc.tile_pool(name="pa", bufs=1) as pa,
                tc.tile_pool(name="pwork", bufs=2) as pw,
                tc.tile_pool(name="pwork1", bufs=1) as pw1,
                tc.tile_pool(name="ps_sc", bufs=5, space="PSUM") as ps_sc,
                tc.tile_pool(name="ps_av", bufs=1, space="PSUM") as ps_av,
                tc.tile_pool(name="ps_tp", bufs=1, space="PSUM") as ps_tp,
            ):
                # dn loads on SWDGE (gpsimd) queues: keeps the sync queue
                # free so phase-A PE work isn't gated behind this drain.
                dn_s = []
                for i in range(NC):
                    t = pa.tile([P, D], F32R, name=f"dn{i}")
                    nc.gpsimd.dma_start(t[:], dn_r[:, i, :])
                    dn_s.append(t)

                # Software pipeline: the next chunk's score matmuls are
                # emitted into the softmax-latency stall of the current
                # chunk, using a 5-slot rotating score-PSUM pool.
                scs = {}
                mx4s = {}
                nm3s = {}

                def emit_scores_mm(lc, ts):
                    lq_sl = slice(lc * P, (lc + 1) * P)
                    if lc not in mx4s:
                        mx4s[lc] = pw.tile([P, NT], F32, name="mx4")
                    for ec in range(EC):
                        for t in ts:
                            if (lc, t) not in scs:
                                scs[(lc, t)] = ps_sc.tile([P, 512], F32, name="sc")
                            nc.tensor.matmul(
                                scs[(lc, t)][:],
                                aqT[ec][:, lq_sl],
                                dT_t[ec][:, t * 512 : (t + 1) * 512],
                                start=(ec == 0),
                                stop=(ec == EC - 1),
                            )

                def emit_scores_red(lc, ts):
                    for t in ts:
                        # add the per-doc bias row, then rowmax
                        nc.vector.tensor_tensor(
                            scs[(lc, t)][:],
                            scs[(lc, t)][:],
                            t3b[:, t * 512 : (t + 1) * 512],
                            mybir.AluOpType.add,
                        )
                        nc.vector.reduce_max(
                            mx4s[lc][:, t : t + 1], scs[(lc, t)][:], axis=AX.X
                        )
                    if ts[-1] == NT - 1:
                        # partial max over t0..2; final combine at chunk head
                        nm3 = pw.tile([P, 1], F32, name="nm3")
                        nc.vector.reduce_max(
                            nm3[:], mx4s[lc][:, 0 : NT - 1], axis=AX.X
                        )
                        nm3s[lc] = nm3

                def emit_scores(lc, ts):
                    emit_scores_mm(lc, ts)
                    emit_scores_red(lc, ts)

                emit_scores(0, [0, 1])
                emit_scores(0, [2, 3])
                for lc in range(LC):
                    lq_sl = slice(lc * P, (lc + 1) * P)
                    mx4 = mx4s.pop(lc)
                    nm3 = nm3s.pop(lc)
                    ls8 = pw.tile([P, 2 * NT], F32, name="ls8")
                    negmax = pw.tile([P, 1], F32, name="negmax")
                    nc.vector.tensor_tensor(
                        mx_all[:, lc : lc + 1],
                        nm3[:],
                        mx4[:, NT - 1 : NT],
                        mybir.AluOpType.max,
                    )
                    nc.vector.tensor_scalar_mul(
                        negmax[:], mx_all[:, lc : lc + 1], -1.0
                    )
                    if lc + 1 < LC:
                        emit_scores_mm(lc + 1, [0, 1])
                    # per 512-group: exp -> transpose -> AV, interleaved
                    av = ps_av.tile([P, D], F32, name="av")
                    for g in range(NT):
                        sc = scs.pop((lc, g))
                        probs_h = [
                            pw1.tile([P, 256], F32R, name=f"probs{g}_{h}")
                            for h in range(2)
                        ]
                        for h in range(2):
                            nc.scalar.activation(
                                probs_h[h][:],
                                sc[:, h * 256 : (h + 1) * 256],
                                ACT.Exp,
                                bias=negmax[:],
                                accum_out=ls8[:, 2 * g + h : 2 * g + h + 1],
                            )
                        tp = ps_tp.tile([P, 512], F32R, name="tp")
                        for j in range(4):
                            nc.tensor.transpose(
                                tp[:, j * P : (j + 1) * P],
                                probs_h[j // 2][:, (j % 2) * P : (j % 2 + 1) * P],
                                ident[:],
                            )
                        probsT = pw.tile([P, 4, P], F32R, name=f"probsT{g}")
                        nc.vector.tensor_copy(probsT[:], tp[:])
                        for j in range(4):
                            nn = g * 4 + j
                            for dh in range(2):
                                nc.tensor.matmul(
                                    av[:, dh * 512 : (dh + 1) * 512],
                                    probsT[:, j, :],
                                    dn_s[nn][:, dh * 512 : (dh + 1) * 512],
                                    start=(nn == 0),
                                    stop=(nn == NC - 1),
                                )
                        if lc + 1 < LC:
                            if g == 0:
                                emit_scores_red(lc + 1, [0, 1])
                                emit_scores_mm(lc + 1, [2, 3])
                            elif g == 2:
                                emit_scores_red(lc + 1, [2, 3])
                    nc.vector.reduce_sum(
                        ls_all[:, lc : lc + 1], ls8[:], axis=AX.X
                    )
                    num_t = pw1.tile([P, D], F32, name="num_t")
                    nc.scalar.activation(num_t[:], av[:], ACT.Copy)
                    nc.sync.dma_start(num.ap()[lq_sl, :], num_t[:])

            nc.sync.dma_start(mx.ap()[:, :], mx_all[:])
            nc.sync.dma_start(ls.ap()[:, :], ls_all[:])

    nc.compile()
    return nc


def _prep_inputs(query, documents, Wq, bq, Wk, bk):
    query = np.asarray(query, dtype=np.float32)
    documents = np.asarray(documents, dtype=np.float32)
    Wq64 = np.asarray(Wq, np.float64)
    Wk64 = np.asarray(Wk, np.float64)
    bq64 = np.asarray(bq, np.float64)
    wqk = np.ascontiguousarray((Wq64.T @ Wk64).astype(np.float32))
    w = (Wk64.T @ bq64).astype(np.float32)  # [D] per-doc bias vector
    wrep = np.ascontiguousarray(
        np.broadcast_to(w.reshape(DC, P).T[:, :, None], (P, DC, P))
    ).astype(np.float32)
    in_maps = []
    for b in range(B):
        qTh = np.ascontiguousarray(query[b].T)
        for h in range(2):
            d_slice = documents[b, h * N2 : (h + 1) * N2]
            in_maps.append(
                {
                    "qT": qTh,
                    "dT": np.ascontiguousarray(d_slice.T),
                    "dn": np.ascontiguousarray(d_slice),
                    "wqk": wqk,
                    "wrep": wrep,
                }
            )
    return in_maps


def _merge(results):
    out = np.empty((B, LQ, D), dtype=np.float32)
    for b in range(B):
        r0, r1 = results[2 * b], results[2 * b + 1]
        m0 = np.asarray(r0["mx"]).T.reshape(LQ).astype(np.float64)
        m1 = np.asarray(r1["mx"]).T.reshape(LQ).astype(np.float64)
        l0 = np.asarray(r0["ls"]).T.reshape(LQ).astype(np.float64)
        l1 = np.asarray(r1["ls"]).T.reshape(LQ).astype(np.float64)
        n0 = np.asarray(r0["num"]).astype(np.float64)
        n1 = np.asarray(r1["num"]).astype(np.float64)
        m = np.maximum(m0, m1)
        a0 = np.exp(m0 - m)
        a1 = np.exp(m1 - m)
        denom = a0 * l0 + a1 * l1
        out[b] = ((a0[:, None] * n0 + a1[:, None] * n1) / denom[:, None]).astype(
            np.float32
        )
    return out


def run(inputs, trace=False, trace_kwargs=None):
    """Run the SPMD kernel; returns (output, BassKernelResults)."""
    if "nc" not in _CACHE:
        _CACHE["nc"] = build_nc()
    nc = _CACHE["nc"]
    in_maps = _prep_inputs(**inputs)
    kw = {}
    if trace:
        kw["trace"] = True
        kw.update(trace_kwargs or {})
    res = run_bass_kernel_spmd(nc, in_maps, core_ids=list(range(8)), **kw)
    return _merge(res.results), res


def kernel(**inputs) -> np.ndarray:
    out, _ = run(inputs)
    return out


# revision 26
# speedup vs baseline: 1.1721x; 1.0107x over previous
"""Trainium2 Bass kernel for nn_AttentionMechanism (B=4, LQ=1024, ND=4096, D=1024).

Sharding: batch (4) x num_docs (2) -> 8 cores. Core c handles batch c//2 and
doc half c%2 (2048 docs).

Algebraic restructuring (exact up to float rounding):
  scores = (x@Wq.T + bq) @ (docs@Wk.T + bk).T
         = x @ (Wq.T@Wk) @ docs.T + [x@(Wq.T@bk)]_per-query + [docs@(Wk.T@bq)]_per-doc + bq.bk
Softmax over docs is invariant to per-query constants, so only
  scores' = x @ Wqk @ docs.T + t3[n],   Wqk = Wq.T@Wk (host),  t3 = docs @ (Wk.T@bq)
is needed — the K-projection (the largest matmul block) disappears entirely and
raw docs.T is the scores operand. Both per-core partials of a batch drop the
same per-query constants, so the host-side softmax-stat merge is unaffected.

Per core:
  aqT = Wqk.T-chunks @ queryT     [d', lq]  (fp32r, d' on partitions)
  t3b = broadcast(docs @ w)       [128, n]  (via replicated-w matmul)
  s   = aqT.T @ docsT + t3        [lq, n]   per 128-row chunk, PSUM
  m   = rowmax(s); p = exp(s - m); l = rowsum(p)
  num = p @ docs                  [lq, d]
Host merges the two doc-halves per batch (softmax-stat rescale) and divides.

All heavy matmuls run as float32r (TF32-like, full PE rate).
"""

import sys

if "/opt/trn_rl_repo" not in sys.path:
    sys.path.insert(0, "/opt/trn_rl_repo")

import numpy as np

import concourse.bass as bass  # noqa: F401
import concourse.mybir as mybir
from concourse import bacc
from concourse.tile import TileContext
from concourse.masks import make_identity
from concourse.bass_utils import run_bass_kernel_spmd

P = 128
B, LQ, ND, D = 4, 1024, 4096, 1024
N2 = ND // 2  # docs per core
EC = D // P  # 8 contraction chunks (d')
DC = D // P  # 8 contraction chunks (d)
LC = LQ // P  # 8 lq-chunks
NC = N2 // P  # 16 n-chunks
NT = N2 // 512  # 4 n-tiles of 512

F32 = mybir.dt.float32
F32R = mybir.dt.float32r
ACT = mybir.ActivationFunctionType
AX = mybir.AxisListType

_CACHE = {}


def build_nc():
    nc = bacc.Bacc("TRN2", target_bir_lowering=False)

    qT = nc.dram_tensor("qT", [D, LQ], F32, kind="ExternalInput")
    dT = nc.dram_tensor("dT", [D, N2], F32, kind="ExternalInput")
    dn = nc.dram_tensor("dn", [N2, D], F32, kind="ExternalInput")
    wqk = nc.dram_tensor("wqk", [D, D], F32, kind="ExternalInput")
    wrep = nc.dram_tensor("wrep", [P, DC, P], F32, kind="ExternalInput")

    num = nc.dram_tensor("num", [LQ, D], F32, kind="ExternalOutput")
    mx = nc.dram_tensor("mx", [P, LC], F32, kind="ExternalOutput")
    ls = nc.dram_tensor("ls", [P, LC], F32, kind="ExternalOutput")

    qT_r = qT.ap().rearrange("(dc p) l -> p dc l", p=P).bitcast(F32R)
    dT_r = dT.ap().rearrange("(dc p) n -> p dc n", p=P).bitcast(F32R)
    dn_r = dn.ap().rearrange("(nc p) d -> p nc d", p=P).bitcast(F32R)
    wqk_r = wqk.ap().rearrange("(dc p) e -> p dc e", p=P).bitcast(F32R)
    wrep_r = wrep.ap().bitcast(F32R)

    with TileContext(nc) as tc:
        with (
            tc.tile_pool(name="const", bufs=1) as cpool,
            tc.tile_pool(name="stats", bufs=1) as spool,
            tc.tile_pool(name="dTp", bufs=1) as dT_pool,
            tc.tile_pool(name="aqTp", bufs=1) as aqT_pool,
            tc.tile_pool(name="t3p", bufs=1) as t3_pool,
        ):
            ident32 = cpool.tile([P, P], F32)
            make_identity(nc, ident32[:])
            ident = cpool.tile([P, P], F32R)
            nc.vector.tensor_copy(ident[:], ident32[:])

            mx_all = spool.tile([P, LC], F32)
            ls_all = spool.tile([P, LC], F32)

            aqT = [aqT_pool.tile([P, LQ], F32R, name=f"aqT{ec}") for ec in range(EC)]
            t3b = t3_pool.tile([P, N2], F32)
            dT_t = [dT_pool.tile([P, N2], F32R, name=f"dTt{dc}") for dc in range(DC)]

            # ---- Phase P: aqT[d', lq] = Wqk.T-chunks @ queryT; t3 row ----
            with (
                tc.tile_pool(name="pp", bufs=1) as pp,
                tc.tile_pool(name="psp", bufs=4, space="PSUM") as psp,
                tc.tile_pool(name="ps3", bufs=4, space="PSUM") as ps3,
            ):
                wqk_t, qT_t = [], []
                for dc in range(DC):
                    w = pp.tile([P, D], F32R, name=f"wqk{dc}")
                    q = pp.tile([P, LQ], F32R, name=f"qTt{dc}")
                    nc.sync.dma_start(w[:], wqk_r[:, dc, :])
                    nc.sync.dma_start(q[:], qT_r[:, dc, :])
                    wqk_t.append(w)
                    qT_t.append(q)
                wrep_s = pp.tile([P, DC, P], F32R, name="wrep")
                nc.sync.dma_start(wrep_s[:], wrep_r)
                for dc in range(DC):
                    nc.sync.dma_start(dT_t[dc][:], dT_r[:, dc, :])

                for ec in range(EC):
                    pss = [psp.tile([P, 512], F32, name="psp") for t in range(2)]
                    for dc in range(DC):
                        for t in range(2):
                            nc.tensor.matmul(
                                pss[t][:],
                                wqk_t[dc][:, ec * P : (ec + 1) * P],
                                qT_t[dc][:, t * 512 : (t + 1) * 512],
                                start=(dc == 0),
                                stop=(dc == DC - 1),
                            )
                    for t in range(2):
                        nc.scalar.activation(
                            aqT[ec][:, t * 512 : (t + 1) * 512],
                            pss[t][:],
                            ACT.Identity,
                            bias=0.0,
                        )

                # t3 broadcast row: every partition gets t3[n] (w replicated
                # as the stationary operand's columns)
                ps3t = [ps3.tile([P, 512], F32, name="ps3") for t in range(NT)]
                for dc in range(DC):
                    for t in range(NT):
                        nc.tensor.matmul(
                            ps3t[t][:],
                            wrep_s[:, dc, :],
                            dT_t[dc][:, t * 512 : (t + 1) * 512],
                            start=(dc == 0),
                            stop=(dc == DC - 1),
                        )
                for t in range(NT):
                    nc.scalar.activation(
                        t3b[:, t * 512 : (t + 1) * 512],
                        ps3t[t][:],
                        ACT.Copy,
                    )

            # ---- Phase A: attention per 128-query chunk ----
            with (
                tc.tile_pool(name="pa", bufs=1) as pa,
                tc.tile_pool(name="pwork", bufs=2) as pw,
                tc.tile_pool(name="pwork1", bufs=1) as pw1,
                tc.tile_pool(name="ps_sc", bufs=5, space="PSUM") as ps_sc,
                tc.tile_pool(name="ps_av", bufs=1, space="PSUM") as ps_av,
                tc.tile_pool(name="ps_tp", bufs=1, space="PSUM") as ps_tp,
            ):
                # dn loads on SWDGE (gpsimd) queues: keeps the sync queue
                # free so phase-A PE work isn't gated behind this drain.
                dn_s = []
                for i in range(NC):
                    t = pa.tile([P, D], F32R, name=f"dn{i}")
                    nc.gpsimd.dma_start(t[:], dn_r[:, i, :])
                    dn_s.append(t)

                # Software pipeline: the next chunk's score matmuls are
                # emitted into the softmax-latency stall of the current
                # chunk, using a 5-slot rotating score-PSUM pool.
                scs = {}
                mx4s = {}
                nm3s = {}

                def emit_scores_mm(lc, ts):
                    lq_sl = slice(lc * P, (lc + 1) * P)
                    if lc not in mx4s:
                        mx4s[lc] = pw.tile([P, NT], F32, name="mx4")
                    for ec in range(EC):
                        for t in ts:
                            if (lc, t) not in scs:
                                scs[(lc, t)] = ps_sc.tile([P, 512], F32, name="sc")
                            nc.tensor.matmul(
                                scs[(lc, t)][:],
                                aqT[ec][:, lq_sl],
                                dT_t[ec][:, t * 512 : (t + 1) * 512],
                                start=(ec == 0),
                                stop=(ec == EC - 1),
                            )

                def emit_scores_red(lc, ts):
                    for t in ts:
                        # add the per-doc bias row, then rowmax
                        nc.vector.tensor_tensor(
                            scs[(lc, t)][:],
                            scs[(lc, t)][:],
                            t3b[:, t * 512 : (t + 1) * 512],
                            mybir.AluOpType.add,
                        )
                        nc.vector.reduce_max(
                            mx4s[lc][:, t : t + 1], scs[(lc, t)][:], axis=AX.X
                        )
                    if ts[-1] == NT - 1:
                        # partial max over t0..2; final combine at chunk head
                        nm3 = pw.tile([P, 1], F32, name="nm3")
                        nc.vector.reduce_max(
                            nm3[:], mx4s[lc][:, 0 : NT - 1], axis=AX.X
                        )
                        nm3s[lc] = nm3

                def emit_scores(lc, ts):
                    emit_scores_mm(lc, ts)
                    emit_scores_red(lc, ts)

                emit_scores(0, [0, 1])
                emit_scores(0, [2, 3])
                for lc in range(LC):
                    lq_sl = slice(lc * P, (lc + 1) * P)
                    mx4 = mx4s.pop(lc)
                    nm3 = nm3s.pop(lc)
                    ls8 = pw.tile([P, 2 * NT], F32, name="ls8")
                    negmax = pw.tile([P, 1], F32, name="negmax")
                    # nm3 (max over tiles 0..2) is a valid softmax shift:
                    # tile 3's excess is bounded (fp32 exp headroom is vast) and
                    # the host merge is exact for any consistent per-core max.
                    # This removes rm(t3) from the exp critical chain.
                    nc.vector.tensor_copy(mx_all[:, lc : lc + 1], nm3[:])
                    nc.vector.tensor_scalar_mul(negmax[:], nm3[:], -1.0)
                    if lc + 1 < LC:
                        emit_scores_mm(lc + 1, [0, 1])
                    # per 512-group: exp -> transpose -> AV, interleaved
                    av = ps_av.tile([P, D], F32, name="av")
                    for g in range(NT):
                        sc = scs.pop((lc, g))
                        probs_h = [
                            pw1.tile([P, 256], F32R, name=f"probs{g}_{h}")
                            for h in range(2)
                        ]
                        for h in range(2):
                            nc.scalar.activation(
                                probs_h[h][:],
                                sc[:, h * 256 : (h + 1) * 256],
                                ACT.Exp,
                                bias=negmax[:],
                                accum_out=ls8[:, 2 * g + h : 2 * g + h + 1],
                            )
                        tp = ps_tp.tile([P, 512], F32R, name="tp")
                        for j in range(4):
                            nc.tensor.transpose(
                                tp[:, j * P : (j + 1) * P],
                                probs_h[j // 2][:, (j % 2) * P : (j % 2 + 1) * P],
                                ident[:],
                            )
                        probsT = pw.tile([P, 4, P], F32R, name=f"probsT{g}")
                        nc.vector.tensor_copy(probsT[:], tp[:])
                        for j in range(4):
                            nn = g * 4 + j
                            for dh in range(2):
                                nc.tensor.matmul(
                                    av[:, dh * 512 : (dh + 1) * 512],
                                    probsT[:, j, :],
                                    dn_s[nn][:, dh * 512 : (dh + 1) * 512],
                                    start=(nn == 0),
                                    stop=(nn == NC - 1),
                                )
                        if lc + 1 < LC:
                            if g == 0:
                                emit_scores_red(lc + 1, [0, 1])
                                emit_scores_mm(lc + 1, [2, 3])
                            elif g == 2:
                                emit_scores_red(lc + 1, [2, 3])
                    nc.vector.reduce_sum(
                        ls_all[:, lc : lc + 1], ls8[:], axis=AX.X
                    )
                    num_t = pw1.tile([P, D], F32, name="num_t")
                    nc.scalar.activation(num_t[:], av[:], ACT.Copy)
                    nc.sync.dma_start(num.ap()[lq_sl, :], num_t[:])

            nc.sync.dma_start(mx.ap()[:, :], mx_all[:])
            nc.sync.dma_start(ls.ap()[:, :], ls_all[:])

    nc.compile()
    return nc


def _prep_inputs(query, documents, Wq, bq, Wk, bk):
    query = np.asarray(query, dtype=np.float32)
    documents = np.asarray(documents, dtype=np.float32)
    Wq64 = np.asarray(Wq, np.float64)
    Wk64 = np.asarray(Wk, np.float64)
    bq64 = np.asarray(bq, np.float64)
    wqk = np.ascontiguousarray((Wq64.T @ Wk64).astype(np.float32))
    w = (Wk64.T @ bq64).astype(np.float32)  # [D] per-doc bias vector
    wrep = np.ascontiguousarray(
        np.broadcast_to(w.reshape(DC, P).T[:, :, None], (P, DC, P))
    ).astype(np.float32)
    in_maps = []
    for b in range(B):
        qTh = np.ascontiguousarray(query[b].T)
        for h in range(2):
            d_slice = documents[b, h * N2 : (h + 1) * N2]
            in_maps.append(
                {
                    "qT": qTh,
                    "dT": np.ascontiguousarray(d_slice.T),
                    "dn": np.ascontiguousarray(d_slice),
                    "wqk": wqk,
                    "wrep": wrep,
                }
            )
    return in_maps


def _merge(results):
    out = np.empty((B, LQ, D), dtype=np.float32)
    for b in range(B):
        r0, r1 = results[2 * b], results[2 * b + 1]
        m0 = np.asarray(r0["mx"]).T.reshape(LQ).astype(np.float64)
        m1 = np.asarray(r1["mx"]).T.reshape(LQ).astype(np.float64)
        l0 = np.asarray(r0["ls"]).T.reshape(LQ).astype(np.float64)
        l1 = np.asarray(r1["ls"]).T.reshape(LQ).astype(np.float64)
        n0 = np.asarray(r0["num"]).astype(np.float64)
        n1 = np.asarray(r1["num"]).astype(np.float64)
        m = np.maximum(m0, m1)
        a0 = np.exp(m0 - m)
        a1 = np.exp(m1 - m)
        denom = a0 * l0 + a1 * l1
        out[b] = ((a0[:, None] * n0 + a1[:, None] * n1) / denom[:, None]).astype(
            np.float32
        )
    return out


def run(inputs, trace=False, trace_kwargs=None):
    """Run the SPMD kernel; returns (output, BassKernelResults)."""
    if "nc" not in _CACHE:
        _CACHE["nc"] = build_nc()
    nc = _CACHE["nc"]
    in_maps = _prep_inputs(**inputs)
    kw = {}
    if trace:
        kw["trace"] = True
        kw.update(trace_kwargs or {})
    res = run_bass_kernel_spmd(nc, in_maps, core_ids=list(range(8)), **kw)
    return _merge(res.results), res


def kernel(**inputs) -> np.ndarray:
    out, _ = run(inputs)
    return out
